# Initial kernel scaffold
#
"""Trainium2 Bass kernel for nn_CrossAttention (B=2, Lq=Lkv=2048, E=1024, H=16, D=64).

Sharding: tensor-parallel over heads. Each of the 8 cores owns 2 heads
(a 128-wide slice of the QKV projection output and the matching 128
columns of Wo). Per core:

  phase P: Q^T/K^T/V^T projections (contraction over E in 8 chunks of
           128, fp32r matmuls at full PE rate), biases fused into the
           PSUM->SBUF copy on ScalarE.
  phase T: V^T -> V via PE transposes; V stored as [kpart, chunk,
           [1|h0|1|h1]] so a ones column rides along as stationary
           column 0, making each context matmul also produce the
           softmax denominator in PSUM row 0.
  phase A: flash-style attention per (batch, 512-wide q tile):
           scores^T = K^T.T @ Q^T with 64-row PE tiling (head0 on
           partitions 0-63, head1 on 64-127, concurrent); exp+mask via
           one ScalarE activation (scale=1/8, per-partition additive
           mask bias) straight from PSUM; context accumulated over the
           16 k chunks into 4 PSUM banks (2 row-tiles x 2 heads);
           denominator division via reciprocal + K=1 broadcast matmul;
           SBUF->SBUF DMA assembles ctx into [128 j, t] layout.
  phase O: out^T partial = Wo_c^T.T @ ctx, written to DRAM; host sums
           the 8 partials (the row-parallel Wo all-reduce).
"""

import sys

if "/opt/trn_rl_repo" not in sys.path:
    sys.path.insert(0, "/opt/trn_rl_repo")

import numpy as np

import concourse.tile as tile
from concourse import bacc, mybir
from concourse.bass_utils import run_bass_kernel_spmd
from concourse.masks import make_identity

F32 = mybir.dt.float32
F32R = mybir.dt.float32r
AF = mybir.ActivationFunctionType

N_CORES = 8
B, LQ, LKV, E, H, D = 2, 2048, 2048, 1024, 16, 64
HC = H // N_CORES  # heads per core = 2
JC = HC * D  # feature slice per core = 128
T = B * LQ  # 4096 tokens
NEC = E // 128  # 8 e-chunks
NTT = T // 512  # 8 token tiles of 512
NQT = LQ // 512  # 4 q tiles per batch
NKT = LKV // 128  # 16 k chunks per batch
NOC = E // 128  # 8 output chunks

_NC_CACHE = {}


def build(reps=None, phases="PTAO"):
    key = (reps or 0, phases)
    if key in _NC_CACHE:
        return _NC_CACHE[key]
    nc = bacc.Bacc("TRN2", target_bir_lowering=False, debug=False, num_devices=N_CORES)

    xqT = nc.dram_tensor("xqT", [E, T], F32R, kind="ExternalInput").ap()
    xkT = nc.dram_tensor("xkT", [E, T], F32R, kind="ExternalInput").ap()
    wqT = nc.dram_tensor("wqT", [E, JC], F32R, kind="ExternalInput").ap()
    wkT = nc.dram_tensor("wkT", [E, JC], F32R, kind="ExternalInput").ap()
    wvT = nc.dram_tensor("wvT", [E, JC], F32R, kind="ExternalInput").ap()
    woT = nc.dram_tensor("woT", [JC, E], F32R, kind="ExternalInput").ap()
    bqd = nc.dram_tensor("bq", [JC, 1], F32, kind="ExternalInput").ap()
    bkd = nc.dram_tensor("bk", [JC, 1], F32, kind="ExternalInput").ap()
    bvd = nc.dram_tensor("bv", [JC, 1], F32, kind="ExternalInput").ap()
    bod = nc.dram_tensor("bo", [NOC, 128], F32, kind="ExternalInput").ap()
    mbd = nc.dram_tensor("mb", [B, NKT, 128], F32, kind="ExternalInput").ap()
    outT = nc.dram_tensor("outT", [E, T], F32, kind="ExternalOutput").ap()

    from contextlib import nullcontext

    with tile.TileContext(nc) as tc, nc.allow_low_precision(reason="fp32r matmuls"):
        with tc.For_i(0, reps, 1) if reps else nullcontext():
         with (
             tc.tile_pool(name="const", bufs=1) as const,
             tc.tile_pool(name="big", bufs=1) as big,
         ):
             # ---- persistent SBUF state ----
             wq_sb = const.tile([128, NEC, JC], F32R, tag="wq")
             nc.sync.dma_start(out=wq_sb, in_=wqT.rearrange("(ec p) j -> p ec j", p=128))
             wk_sb = const.tile([128, NEC, JC], F32R, tag="wk")
             nc.sync.dma_start(out=wk_sb, in_=wkT.rearrange("(ec p) j -> p ec j", p=128))
             wv_sb = const.tile([128, NEC, JC], F32R, tag="wv")
             nc.sync.dma_start(out=wv_sb, in_=wvT.rearrange("(ec p) j -> p ec j", p=128))
             wo_sb = const.tile([128, NOC, 128], F32R, tag="wo")
             nc.sync.dma_start(out=wo_sb, in_=woT.rearrange("p (oc o) -> p oc o", oc=NOC))
             bq_sb = const.tile([128, 1], F32, tag="bq")
             nc.sync.dma_start(out=bq_sb, in_=bqd)
             bk_sb = const.tile([128, 1], F32, tag="bk")
             nc.sync.dma_start(out=bk_sb, in_=bkd)
             bv_sb = const.tile([128, 1], F32, tag="bv")
             nc.sync.dma_start(out=bv_sb, in_=bvd)
             bo_sb = const.tile([128, NOC], F32, tag="bo")
             nc.sync.dma_start(out=bo_sb, in_=bod.rearrange("oc o -> o oc"))
             mb_sb = const.tile([128, B, NKT], F32, tag="mb")
             nc.sync.dma_start(out=mb_sb, in_=mbd.rearrange("b kc p -> p b kc"))
             ident = const.tile([128, 128], F32, tag="ident")
             make_identity(nc, ident)
             ones_f = const.tile([1, 65], F32, tag="onesf")
             nc.vector.memset(ones_f, 1.0)
             onesc = const.tile([1, 65], F32R, tag="onesc")
             nc.vector.tensor_copy(onesc, ones_f)
             onecol = const.tile([128, 1], F32, tag="onecol")
             nc.vector.memset(onecol, 1.0)

             qt_sb = big.tile([128, T], F32R, tag="qt")
             kt_sb = big.tile([128, T], F32R, tag="kt")
             vt_sb = big.tile([128, T], F32, tag="vt")
             v_sb = big.tile([128, B * NKT, 130], F32R, tag="v")
             ctx_sb = big.tile([128, NTT, 512], F32R, tag="ctx")

             # ---- phase P: projections ----
             if "P" in phases:
              with (
                 tc.tile_pool(name="xin", bufs=2) as xin,
                 tc.tile_pool(name="pp", bufs=3, space="PSUM") as pp,
             ):
                 for xsrc, wsb, bias, dst, isv in (
                     (xqT, wq_sb, bq_sb, qt_sb, False),
                     (xkT, wk_sb, bk_sb, kt_sb, False),
                     (xkT, wv_sb, bv_sb, vt_sb, True),
                 ):
                     for tt in range(NTT):
                         xt = xin.tile([128, NEC, 512], F32R, tag="xin")
                         nc.sync.dma_start(
                             out=xt,
                             in_=xsrc[:, tt * 512 : (tt + 1) * 512].rearrange(
                                 "(ec p) t -> p ec t", p=128
                             ),
                         )
                         pt = pp.tile([128, 512], F32, tag="pp")
                         for ec in range(NEC):
                             nc.tensor.matmul(
                                 pt,
                                 wsb[:, ec, :],
                                 xt[:, ec, :],
                                 start=(ec == 0),
                                 stop=(ec == NEC - 1),
                             )
                         nc.scalar.activation(
                             out=dst[:, tt * 512 : (tt + 1) * 512],
                             in_=pt,
                             func=AF.Identity,
                             bias=bias,
                             scale=1.0,
                         )

             # ---- phase T: V transpose into [kpart, chunk, [1|h0|1|h1]] ----
             if "T" in phases:
              with tc.tile_pool(name="tp", bufs=3, space="PSUM") as tp:
                 for gc in range(B * NKT):
                     tpt = tp.tile([128, 128], F32, tag="tp")
                     nc.tensor.transpose(
                         tpt, vt_sb[:, gc * 128 : (gc + 1) * 128], ident
                     )
                     nc.vector.tensor_copy(v_sb[:, gc, 1:65], tpt[:, 0:64])
                     nc.vector.tensor_copy(v_sb[:, gc, 66:130], tpt[:, 64:128])
                     nc.vector.tensor_copy(v_sb[:, gc, 0:1], onecol)
                     nc.vector.tensor_copy(v_sb[:, gc, 65:66], onecol)

             # ---- phase A: attention ----
             if "A" in phases:
              with (
                 tc.tile_pool(name="attps", bufs=2, space="PSUM") as attps,
                 tc.tile_pool(name="cxps", bufs=1, space="PSUM") as cxps,
                 tc.tile_pool(name="expm", bufs=3) as expm,
                 tc.tile_pool(name="dv", bufs=2) as dv,
             ):
                 for b in range(B):
                     for qt in range(NQT):
                         q0 = b * LQ + qt * 512
                         cxs = [
                             cxps.tile([65, 512], F32, tag=f"cx{i}", name=f"cx{i}_{b}_{qt}")
                             for i in range(4)
                         ]
                         for kt in range(NKT):
                             k0 = b * LKV + kt * 128
                             sct = attps.tile([128, 2, 512], F32, tag="sc")
                             nc.tensor.matmul(
                                 sct[:, 0, :],
                                 kt_sb[0:64, k0 : k0 + 128],
                                 qt_sb[0:64, q0 : q0 + 512],
                                 start=True,
                                 stop=True,
                             )
                             nc.tensor.matmul(
                                 sct[:, 1, :],
                                 kt_sb[64:128, k0 : k0 + 128],
                                 qt_sb[64:128, q0 : q0 + 512],
                                 start=True,
                                 stop=True,
                             )
                             emt = expm.tile([128, 2, 512], F32R, tag="expm")
                             nc.scalar.activation(
                                 out=emt.rearrange("p a t -> p (a t)"),
                                 in_=sct.rearrange("p a t -> p (a t)"),
                                 func=AF.Exp,
                                 bias=mb_sb[:, b, kt : kt + 1],
                                 scale=0.125,
                             )
                             st, sp = (kt == 0), (kt == NKT - 1)
                             gc = b * NKT + kt
                             nc.tensor.matmul(
                                 cxs[0], v_sb[0:64, gc, 0:65], emt[0:64, 0, :],
                                 start=st, stop=sp,
                             )
                             nc.tensor.matmul(
                                 cxs[1], v_sb[64:128, gc, 0:65], emt[64:128, 0, :],
                                 start=st, stop=sp,
                             )
                             nc.tensor.matmul(
                                 cxs[2], v_sb[0:64, gc, 65:130], emt[0:64, 1, :],
                                 start=st, stop=sp,
                             )
                             nc.tensor.matmul(
                                 cxs[3], v_sb[64:128, gc, 65:130], emt[64:128, 1, :],
                                 start=st, stop=sp,
                             )
                         tt = b * NQT + qt
                         for h in range(HC):
                             cxa, cxb = cxs[2 * h], cxs[2 * h + 1]
                             s1 = dv.tile([65, 512], F32, tag="s1")
                             nc.vector.tensor_copy(s1, cxa)
                             s2 = dv.tile([65, 512], F32, tag="s2")
                             nc.vector.tensor_add(s2, s1, cxb)
                             rr = dv.tile([1, 512], F32R, tag="rr")
                             nc.vector.reciprocal(rr, s2[0:1, :])
                             s2r = dv.tile([65, 512], F32R, tag="s2r")
                             nc.vector.tensor_copy(s2r, s2)
                             bct = attps.tile([65, 512], F32, tag="sc")
                             nc.tensor.matmul(bct, onesc, rr, start=True, stop=True)
                             cs = dv.tile([65, 512], F32R, tag="cs")
                             nc.vector.tensor_mul(cs, s2r, bct)
                             nc.sync.dma_start(
                                 out=ctx_sb[h * 64 : (h + 1) * 64, tt, :],
                                 in_=cs[1:65, :],
                             )

             # ---- phase O: output projection (partial; host sums cores) ----
             if "O" in phases:
              with (
                 tc.tile_pool(name="ops", bufs=3, space="PSUM") as ops,
                 tc.tile_pool(name="outsb", bufs=3) as outsb,
             ):
                 for tt in range(NTT):
                     for oc in range(NOC):
                         opt = ops.tile([128, 512], F32, tag="op")
                         nc.tensor.matmul(
                             opt, wo_sb[:, oc, :], ctx_sb[:, tt, :],
                             start=True, stop=True,
                         )
                         ob = outsb.tile([128, 512], F32, tag="ob")
                         nc.vector.tensor_scalar_add(ob, opt, bo_sb[:, oc : oc + 1])
                         nc.sync.dma_start(
                             out=outT[oc * 128 : (oc + 1) * 128, tt * 512 : (tt + 1) * 512],
                             in_=ob,
                         )

    nc.compile()
    _NC_CACHE[key] = nc
    return nc


def make_in_maps(query, key_value, mask, Wq, bq, Wk, bk, Wv, bv, Wo, bo):
    xqT = np.ascontiguousarray(query.reshape(T, E).T).astype(np.float32)
    xkT = np.ascontiguousarray(key_value.reshape(T, E).T).astype(np.float32)
    mb = np.where(mask != 0, 0.0, -1.0e5).astype(np.float32).reshape(B, NKT, 128)
    in_maps = []
    for c in range(N_CORES):
        sl = slice(c * JC, (c + 1) * JC)
        in_maps.append(
            {
                "xqT": xqT,
                "xkT": xkT,
                "wqT": np.ascontiguousarray(Wq[sl, :].T).astype(np.float32),
                "wkT": np.ascontiguousarray(Wk[sl, :].T).astype(np.float32),
                "wvT": np.ascontiguousarray(Wv[sl, :].T).astype(np.float32),
                "woT": np.ascontiguousarray(Wo[:, sl].T).astype(np.float32),
                "bq": bq[sl].reshape(JC, 1).astype(np.float32),
                "bk": bk[sl].reshape(JC, 1).astype(np.float32),
                "bv": bv[sl].reshape(JC, 1).astype(np.float32),
                # only core 0 adds bo so the host-side partial sum sees it once
                "bo": (
                    bo.reshape(NOC, 128).astype(np.float32)
                    if c == 0
                    else np.zeros((NOC, 128), np.float32)
                ),
                "mb": mb,
            }
        )
    return in_maps


def kernel(query, key_value, mask, Wq, bq, Wk, bk, Wv, bv, Wo, bo):
    nc = build()
    in_maps = make_in_maps(
        np.asarray(query), np.asarray(key_value), np.asarray(mask),
        np.asarray(Wq), np.asarray(bq), np.asarray(Wk), np.asarray(bk),
        np.asarray(Wv), np.asarray(bv), np.asarray(Wo), np.asarray(bo),
    )
    res = run_bass_kernel_spmd(nc, in_maps, list(range(N_CORES)))
    acc = np.zeros((E, T), np.float32)
    for c in range(N_CORES):
        acc += res.results[c]["outT"]
    return np.ascontiguousarray(acc.T).reshape(B, LQ, E).astype(np.float32)



# revision 5
# speedup vs baseline: 1.0116x; 1.0116x over previous
"""Trainium2 Bass kernel for nn_CrossAttention (B=2, Lq=Lkv=2048, E=1024, H=16, D=64).

Sharding: tensor-parallel over heads. Each of the 8 cores owns 2 heads
(a 128-wide slice of the QKV projection output and the matching 128
columns of Wo); the row-parallel Wo all-reduce is a host-side sum of
the 8 bf16 partial outputs.

Per core, single pass with all pools live so phases overlap:

  P(b): per batch, x_q/x_kv loaded once as bf16 [128, ec, 2048] (per-ec
        DMAs so matmuls start early). Q^T/K^T projections (8 e-chunks,
        bf16, biases via DVE tensor_scalar_add on the PSUM->SBUF copy);
        V built directly in [kv, j] layout by flipping the matmul
        (stationary x^T chunk, moving Wv), so no transpose phase.
        V stored [h0 | 1 | h1] with a shared ones column -> each
        context matmul also emits the softmax denominator.
  A(b,qt): per 512-wide q tile: scores^T = K^T.T @ Q^T with 64-row PE
        tiling (both heads concurrent); one Exp activation per kv chunk
        (scale=1/8, per-partition mask bias) straight from PSUM to bf16
        SBUF; h0 context accumulated inline (64-row tile pairs), h1
        deferred over the buffered exp tiles to halve ctx PSUM usage.
        Division: reciprocal_approx_fast on the denominator row + K=1
        broadcast matmul + one DVE multiply per head. h0 lands directly
        in ctx^T partitions 0:64; h1 shifts via a small SBUF->SBUF DMA.
  O(b,qt): out^T partial = Wo_c^T.T @ ctx^T per 128-row chunk, bf16
        PSUM->SBUF copy, DMA to DRAM. Host sums cores and adds
        bo + Wo @ bv.
"""

import sys

if "/opt/trn_rl_repo" not in sys.path:
    sys.path.insert(0, "/opt/trn_rl_repo")

import numpy as np
import ml_dtypes

import concourse.tile as tile
from concourse import bacc, mybir
from concourse.bass_utils import run_bass_kernel_spmd

F32 = mybir.dt.float32
F32R = mybir.dt.float32r
BF16 = mybir.dt.bfloat16
AF = mybir.ActivationFunctionType

N_CORES = 8
B, LQ, LKV, E, H, D = 2, 2048, 2048, 1024, 16, 64
HC = H // N_CORES  # heads per core = 2
JC = HC * D  # feature slice per core = 128
T = B * LQ  # 4096 tokens
NEC = E // 128  # 8 e-chunks
NQT = LQ // 512  # 4 q tiles per batch
NKT = LKV // 128  # 16 kv chunks per batch
NOC = E // 128  # 8 output chunks
NCG = LQ // 512  # 4 projection column groups per batch

_NC_CACHE = {}


def build(reps=None):
    key = reps or 0
    if key in _NC_CACHE:
        return _NC_CACHE[key]
    nc = bacc.Bacc("TRN2", target_bir_lowering=False, debug=False, num_devices=N_CORES)

    xqT = nc.dram_tensor("xqT", [E, T], BF16, kind="ExternalInput").ap()
    xkT = nc.dram_tensor("xkT", [E, T], BF16, kind="ExternalInput").ap()
    wqT = nc.dram_tensor("wqT", [E, JC], BF16, kind="ExternalInput").ap()
    wkT = nc.dram_tensor("wkT", [E, JC], BF16, kind="ExternalInput").ap()
    wvT = nc.dram_tensor("wvT", [E, JC], BF16, kind="ExternalInput").ap()
    woT = nc.dram_tensor("woT", [JC, E], BF16, kind="ExternalInput").ap()
    bqd = nc.dram_tensor("bq", [JC, 1], F32, kind="ExternalInput").ap()
    bkd = nc.dram_tensor("bk", [JC, 1], F32, kind="ExternalInput").ap()
    mbd = nc.dram_tensor("mb", [B, NKT, 128], F32, kind="ExternalInput").ap()
    outT = nc.dram_tensor("outT", [E, T], BF16, kind="ExternalOutput").ap()

    from contextlib import nullcontext

    with tile.TileContext(nc) as tc, nc.allow_low_precision(reason="bf16 attention"):
        with tc.For_i(0, reps, 1) if reps else nullcontext():
         with (
             tc.tile_pool(name="const", bufs=1) as const,
             tc.tile_pool(name="big", bufs=1) as big,
             tc.tile_pool(name="xin", bufs=1) as xin,
             tc.tile_pool(name="expm", bufs=20) as expm,
             tc.tile_pool(name="dv", bufs=2) as dv,
             tc.tile_pool(name="outsb", bufs=3) as outsb,
             tc.tile_pool(name="psc", bufs=2, space="PSUM") as psc,
             tc.tile_pool(name="pcx", bufs=2, space="PSUM") as pcx,
             tc.tile_pool(name="pmm", bufs=2, space="PSUM") as pmm,
         ):
            # ---- persistent SBUF state ----
            wq_sb = const.tile([128, NEC, JC], BF16, tag="wq")
            nc.sync.dma_start(out=wq_sb, in_=wqT.rearrange("(ec p) j -> p ec j", p=128))
            wk_sb = const.tile([128, NEC, JC], BF16, tag="wk")
            nc.sync.dma_start(out=wk_sb, in_=wkT.rearrange("(ec p) j -> p ec j", p=128))
            wv_sb = const.tile([128, NEC, JC], BF16, tag="wv")
            nc.sync.dma_start(out=wv_sb, in_=wvT.rearrange("(ec p) j -> p ec j", p=128))
            wo_sb = const.tile([128, NOC, 128], BF16, tag="wo")
            nc.sync.dma_start(out=wo_sb, in_=woT.rearrange("p (oc o) -> p oc o", oc=NOC))
            bq_sb = const.tile([128, 1], F32, tag="bq")
            nc.sync.dma_start(out=bq_sb, in_=bqd)
            bk_sb = const.tile([128, 1], F32, tag="bk")
            nc.sync.dma_start(out=bk_sb, in_=bkd)
            mb_sb = const.tile([128, B, NKT], F32, tag="mb")
            nc.sync.dma_start(out=mb_sb, in_=mbd.rearrange("b kc p -> p b kc"))
            ones_f = const.tile([128, 65], F32, tag="onesf")
            nc.vector.memset(ones_f, 1.0)
            onesr = const.tile([128, 65], F32R, tag="onesr")
            nc.vector.tensor_copy(onesr, ones_f)

            qt_sb = big.tile([128, T], BF16, tag="qt")
            kt_sb = big.tile([128, T], BF16, tag="kt")
            # V as [kv, gc, [h0 d | 1 | h1 d]]; shared ones column at 64
            v_sb = big.tile([128, B * NKT, 129], BF16, tag="v")
            nc.vector.memset(v_sb[:, :, 64:65], 1.0)
            ctx_sb = big.tile([128, B * NQT, 512], BF16, tag="ctx")

            def phase_p_load(b):
                """DMA x_q / x_kv for batch b, per-ec so compute starts early."""
                xq_t = xin.tile([128, NEC, LQ], BF16, tag="xq", name=f"xq_{b}")
                xk_t = xin.tile([128, NEC, LKV], BF16, tag="xk", name=f"xk_{b}")
                c0 = b * LQ
                for ec in range(NEC):
                    nc.sync.dma_start(
                        out=xq_t[:, ec, :],
                        in_=xqT[ec * 128 : (ec + 1) * 128, c0 : c0 + LQ],
                    )
                    nc.sync.dma_start(
                        out=xk_t[:, ec, :],
                        in_=xkT[ec * 128 : (ec + 1) * 128, c0 : c0 + LKV],
                    )
                return xq_t, xk_t

            def phase_p_chunk(b, cg, xq_t, xk_t):
                """Project one 512-token column group of batch b (Q, K, V)."""
                c0 = b * LQ + cg * 512
                for w_sb, bias, dst, xt in (
                    (wq_sb, bq_sb, qt_sb, xq_t),
                    (wk_sb, bk_sb, kt_sb, xk_t),
                ):
                    pt = pmm.tile([128, 512], F32, tag="mm", name=f"p_{b}_{cg}_{dst.name}")
                    for ec in range(NEC):
                        nc.tensor.matmul(
                            pt,
                            w_sb[:, ec, :],
                            xt[:, ec, cg * 512 : (cg + 1) * 512],
                            start=(ec == 0),
                            stop=(ec == NEC - 1),
                        )
                    nc.vector.tensor_scalar_add(dst[:, c0 : c0 + 512], pt, bias)
                # V direct in [kv, j] layout: stationary x^T chunk, moving Wv
                for k4 in range(4):
                    kc = cg * 4 + k4
                    gc = b * NKT + kc
                    pv = pmm.tile([128, 128], F32, tag="mm", name=f"pv_{b}_{kc}")
                    for ec in range(NEC):
                        nc.tensor.matmul(
                            pv,
                            xk_t[:, ec, kc * 128 : (kc + 1) * 128],
                            wv_sb[:, ec, :],
                            start=(ec == 0),
                            stop=(ec == NEC - 1),
                        )
                    nc.vector.tensor_copy(v_sb[:, gc, 0:64], pv[:, 0:64])
                    nc.vector.tensor_copy(v_sb[:, gc, 65:129], pv[:, 64:128])

            def div_head(cxa, cxb, den_row, ctx_rows, dst_part0, ti, b, qt, h):
                """Sum the two kv-half accumulators, normalize, store ctx^T."""
                s1 = dv.tile([65, 512], F32, tag="s1", name=f"s1_{b}_{qt}_{h}")
                nc.vector.tensor_copy(s1, cxa)
                s = dv.tile([65, 512], F32, tag="s", name=f"s_{b}_{qt}_{h}")
                nc.vector.tensor_add(s, s1, cxb)
                rcp = dv.tile([65, 512], F32, tag="rcp", name=f"rcp_{b}_{qt}_{h}")
                nc.vector.reciprocal_approx_fast(
                    rcp[den_row : den_row + 1, :], s[den_row : den_row + 1, :]
                )
                rcpr = dv.tile([65, 512], F32R, tag="rcpr", name=f"rcpr_{b}_{qt}_{h}")
                nc.vector.tensor_copy(
                    rcpr[den_row : den_row + 1, :], rcp[den_row : den_row + 1, :]
                )
                bct = pmm.tile([65, 512], F32, tag="mm", name=f"bct_{b}_{qt}_{h}")
                nc.tensor.matmul(
                    bct,
                    onesr[den_row : den_row + 1, 0:65],
                    rcpr[den_row : den_row + 1, :],
                    start=True,
                    stop=True,
                )
                r0, r1 = ctx_rows
                if dst_part0 == 0:
                    nc.vector.tensor_mul(
                        ctx_sb[0:64, ti, :], s[r0:r1, :], bct[r0:r1, :]
                    )
                else:
                    cs = dv.tile([65, 512], BF16, tag="cs", name=f"cs_{b}_{qt}_{h}")
                    nc.vector.tensor_mul(cs, s, bct)
                    nc.sync.dma_start(out=ctx_sb[64:128, ti, :], in_=cs[r0:r1, :])

            def phase_a(b, qt):
                ti = b * NQT + qt
                q0 = b * LQ + qt * 512
                cx0a = pcx.tile([65, 512], F32, tag="cx", name=f"cx0a_{b}_{qt}")
                cx0b = pcx.tile([65, 512], F32, tag="cx", name=f"cx0b_{b}_{qt}")
                emts = []
                for kt in range(NKT):
                    k0 = b * LKV + kt * 128
                    gc = b * NKT + kt
                    sct = psc.tile([128, 2, 512], F32, tag="sc", name=f"sc_{b}_{qt}_{kt}")
                    nc.tensor.matmul(
                        sct[:, 0, :],
                        kt_sb[0:64, k0 : k0 + 128],
                        qt_sb[0:64, q0 : q0 + 512],
                        start=True,
                        stop=True,
                    )
                    nc.tensor.matmul(
                        sct[:, 1, :],
                        kt_sb[64:128, k0 : k0 + 128],
                        qt_sb[64:128, q0 : q0 + 512],
                        start=True,
                        stop=True,
                    )
                    emt = expm.tile(
                        [128, 2, 512], BF16, tag="emt", name=f"emt_{b}_{qt}_{kt}"
                    )
                    nc.scalar.activation(
                        out=emt.rearrange("p a t -> p (a t)"),
                        in_=sct.rearrange("p a t -> p (a t)"),
                        func=AF.Exp,
                        bias=mb_sb[:, b, kt : kt + 1],
                        scale=0.125,
                    )
                    emts.append(emt)
                    st, sp = (kt == 0), (kt == NKT - 1)
                    # h0 context inline: 64-row tile pair, kv halves concurrent
                    nc.tensor.matmul(
                        cx0a, v_sb[0:64, gc, 0:65], emt[0:64, 0, :], start=st, stop=sp
                    )
                    nc.tensor.matmul(
                        cx0b, v_sb[64:128, gc, 0:65], emt[64:128, 0, :],
                        start=st, stop=sp,
                    )
                # h0: rows 0:64 = ctx, row 64 = den -> lands at partitions 0:64
                div_head(cx0a, cx0b, 64, (0, 64), 0, ti, b, qt, 0)
                # h1 deferred over the buffered exp tiles
                cx1a = pcx.tile([65, 512], F32, tag="cx", name=f"cx1a_{b}_{qt}")
                cx1b = pcx.tile([65, 512], F32, tag="cx", name=f"cx1b_{b}_{qt}")
                for kt in range(NKT):
                    gc = b * NKT + kt
                    st, sp = (kt == 0), (kt == NKT - 1)
                    nc.tensor.matmul(
                        cx1a, v_sb[0:64, gc, 64:129], emts[kt][0:64, 1, :],
                        start=st, stop=sp,
                    )
                    nc.tensor.matmul(
                        cx1b, v_sb[64:128, gc, 64:129], emts[kt][64:128, 1, :],
                        start=st, stop=sp,
                    )
                # h1: row 0 = den, rows 1:65 = ctx -> shift to partitions 64:128
                div_head(cx1a, cx1b, 0, (1, 65), 64, ti, b, qt, 1)

            def phase_o(b, qt):
                ti = b * NQT + qt
                t0 = b * LQ + qt * 512
                for oc in range(NOC):
                    opt = pmm.tile([128, 512], F32, tag="mm", name=f"o_{b}_{qt}_{oc}")
                    nc.tensor.matmul(
                        opt, wo_sb[:, oc, :], ctx_sb[:, ti, :], start=True, stop=True
                    )
                    ob = outsb.tile([128, 512], BF16, tag="ob", name=f"ob_{b}_{qt}_{oc}")
                    nc.vector.tensor_copy(ob, opt)
                    nc.sync.dma_start(
                        out=outT[oc * 128 : (oc + 1) * 128, t0 : t0 + 512], in_=ob
                    )

            # ---- schedule: P(b0); A/O(b0) with P(b1) chunks interleaved; A/O(b1)
            xq0, xk0 = phase_p_load(0)
            for cg in range(NCG):
                phase_p_chunk(0, cg, xq0, xk0)
            xq1 = xk1 = None
            for qt in range(NQT):
                phase_a(0, qt)
                phase_o(0, qt)
                if qt == 0:
                    xq1, xk1 = phase_p_load(1)
                phase_p_chunk(1, qt, xq1, xk1)
            for qt in range(NQT):
                phase_a(1, qt)
                phase_o(1, qt)

    nc.compile()
    _NC_CACHE[key] = nc
    return nc


def make_in_maps(query, key_value, mask, Wq, bq, Wk, bk, Wv, bv, Wo, bo):
    bf = ml_dtypes.bfloat16
    xqT = np.ascontiguousarray(query.reshape(T, E).T).astype(bf)
    xkT = np.ascontiguousarray(key_value.reshape(T, E).T).astype(bf)
    mb = np.where(mask != 0, 0.0, -1.0e5).astype(np.float32).reshape(B, NKT, 128)
    in_maps = []
    for c in range(N_CORES):
        sl = slice(c * JC, (c + 1) * JC)
        in_maps.append(
            {
                "xqT": xqT,
                "xkT": xkT,
                "wqT": np.ascontiguousarray(Wq[sl, :].T).astype(bf),
                "wkT": np.ascontiguousarray(Wk[sl, :].T).astype(bf),
                "wvT": np.ascontiguousarray(Wv[sl, :].T).astype(bf),
                "woT": np.ascontiguousarray(Wo[:, sl].T).astype(bf),
                "bq": bq[sl].reshape(JC, 1).astype(np.float32),
                "bk": bk[sl].reshape(JC, 1).astype(np.float32),
                "mb": mb,
            }
        )
    return in_maps


def kernel(query, key_value, mask, Wq, bq, Wk, bk, Wv, bv, Wo, bo):
    query = np.asarray(query)
    key_value = np.asarray(key_value)
    mask = np.asarray(mask)
    Wq, bq = np.asarray(Wq), np.asarray(bq)
    Wk, bk = np.asarray(Wk), np.asarray(bk)
    Wv, bv = np.asarray(Wv), np.asarray(bv)
    Wo, bo = np.asarray(Wo), np.asarray(bo)
    nc = build()
    in_maps = make_in_maps(query, key_value, mask, Wq, bq, Wk, bk, Wv, bv, Wo, bo)
    res = run_bass_kernel_spmd(nc, in_maps, list(range(N_CORES)))
    acc = np.zeros((E, T), np.float32)
    for c in range(N_CORES):
        acc += res.results[c]["outT"].astype(np.float32)
    out = np.ascontiguousarray(acc.T).reshape(B, LQ, E)
    # bv folds through attention as a constant: out += Wo @ bv; plus bo
    out += (Wo.astype(np.float64) @ bv.astype(np.float64) + bo.astype(np.float64)).astype(
        np.float32
    )
    return out.astype(np.float32)
